# revision 34
# baseline (speedup 1.0000x reference)
"""Dense dot-product attention (score = Q@V^T, softmax, context = A@V) on 8
TRN2 NeuronCores, batch-parallel: each core owns B/8 = 2 batches.

Per batch on one core (Lq = Lkv = 1024, D = 512, fp32 I/O):
  - Q, V loaded in natural [l, d] layout (fast contiguous DMA).
  - QT/VT ([d, l] - the PE contracts over the partition dim) via PE
    transpose-mode matmuls; PSUM->SBUF copies round to float32r, which runs
    matmuls at 1 cycle/row (4x fp32) with ~13-bit mantissa (measured score
    RMS err 3e-3, 16x better than bf16).
  - S = QT.T @ VT accumulated in PSUM per 128-row q-tile; per-half
    reduce_max(negate) overlaps the second half's matmuls.
  - ACT exp(S - max) -> fp16 E with fused row-sum; one DMA xbar
    block-transpose per q-tile (dest[p, kt, j] = E[j, kt*128+p]).
  - context = (ET.T @ V_fp16) * (1/Z); attn = E * (1/Z).
The q-loop runs the score matmul two tiles ahead of its consumer, and the
NEXT batch's PE transposes are interleaved into the current batch's loop as
filler so softmax-chain latency doesn't drain the PE.
"""
import sys

sys.path.insert(0, "/opt/trn_rl_repo")

import collections
from contextlib import ExitStack

import numpy as np

import concourse.bass as bass
import concourse.tile as tile
from concourse import mybir
from concourse.bass_utils import run_bass_kernel_spmd

F32 = mybir.dt.float32
F32R = mybir.dt.float32r
F16 = mybir.dt.float16

N_CORES = 8
B, LQ, LKV, D = 16, 1024, 1024, 512
BPC = B // N_CORES  # batches per core
NQT = LQ // 128
NKT = LKV // 128
NDT = D // 128


# --- post-Tile pass: hardware wait-slot limits -------------------------------
# Engine instructions carry a single hardware semaphore-wait slot; Tile's
# sem-assigner sometimes emits more. Hoist excess waits onto single-wait NOPs
# spliced immediately before the instruction on the same engine (the NX
# sequencer dispatches in order, so the NOPs block until the sems clear).
_WAIT_LIMITS = collections.defaultdict(lambda: 1)


def _fix_wait_limits(nc):
    n_fixed = 0
    for fn in nc.m.functions:
        for blk in fn.blocks:
            out = []
            for inst in blk.instructions:
                limit = _WAIT_LIMITS[type(inst).__name__]
                si = inst.sync_info
                if si is not None and si.on_wait and len(si.on_wait) > limit:
                    hoist = list(si.on_wait)[: len(si.on_wait) - limit]
                    keep = list(si.on_wait)[len(si.on_wait) - limit :]
                    for i, w in enumerate(hoist):
                        out.append(
                            mybir.InstNoOp(
                                name=f"{inst.name}-waitnop{i}",
                                engine=inst.engine,
                                sync_info=mybir.SyncInfo(on_wait=[w], on_update=[]),
                                bass_nofuse=True,
                            )
                        )
                    inst.sync_info = mybir.SyncInfo(
                        on_wait=keep, on_update=list(si.on_update or [])
                    )
                    n_fixed += 1
                out.append(inst)
            blk.instructions = out
    return n_fixed


def build():
    nc = bass.Bass("TRN2", target_bir_lowering=False, debug=False)
    q = nc.dram_tensor("query", [BPC, LQ, D], F32, kind="ExternalInput").ap()
    v = nc.dram_tensor("value", [BPC, LKV, D], F32, kind="ExternalInput").ap()
    iden = nc.dram_tensor("iden", [128, 128], F32, kind="ExternalInput").ap()
    ctx_out = nc.dram_tensor("context", [BPC, LQ, D], F32, kind="ExternalOutput").ap()
    attn_out = nc.dram_tensor("attn", [BPC, LQ, LKV], F32, kind="ExternalOutput").ap()

    with ExitStack() as ctx:
        tc = ctx.enter_context(tile.TileContext(nc))
        singles = ctx.enter_context(tc.tile_pool(name="singles", bufs=1))
        iop = ctx.enter_context(tc.tile_pool(name="io", bufs=2))
        tp = ctx.enter_context(tc.tile_pool(name="tp", bufs=2))
        ep = ctx.enter_context(tc.tile_pool(name="ep", bufs=2))
        sp = ctx.enter_context(tc.tile_pool(name="sp", bufs=8))
        pss = ctx.enter_context(tc.tile_pool(name="pss", bufs=3, space="PSUM"))
        psc = ctx.enter_context(tc.tile_pool(name="psc", bufs=2, space="PSUM"))

        ident = singles.tile([128, 128], F32)
        nc.sync.dma_start(ident[:], iden)

        # all loads issued up front (sync ring) so they never queue behind
        # stores; per-half so the first transposes can start early
        qn, vn = {}, {}
        for b in range(BPC):
            qn[b] = iop.tile([128, NQT, D], F32, tag="qn", name=f"qn{b}")
            vn[b] = iop.tile([128, NKT, D], F32, tag="vn", name=f"vn{b}")
        for b in range(BPC):
            for g in range(2):
                nc.sync.dma_start(
                    vn[b][:, g * 4 : (g + 1) * 4, :],
                    v[b].rearrange("(t p) d -> p t d", p=128)[:, g * 4 : (g + 1) * 4, :],
                )
            for g in range(2):
                nc.sync.dma_start(
                    qn[b][:, g * 4 : (g + 1) * 4, :],
                    q[b].rearrange("(t p) d -> p t d", p=128)[:, g * 4 : (g + 1) * 4, :],
                )

        qt, vt, vh = {}, {}, {}
        copy_eng = [0]

        def alloc_transposed(b):
            vh[b] = tp.tile([128, NKT, D], F16, tag="vh", name=f"vh{b}")
            qt[b] = tp.tile([128, NDT, LQ], F32R, tag="qt", name=f"qt{b}")
            vt[b] = tp.tile([128, NDT, LKV], F32R, tag="vt", name=f"vt{b}")

        def transpose_groups(b):
            """Thunks: each emits one PE-transpose group (4 blocks) + its
            PSUM->SBUF rounding copy; final thunk casts V to fp16."""
            out = []
            for src_sel, g in ((1, 0), (1, 1), (0, 0), (0, 1)):
                if True:
                    for dt in range(NDT):
                        def emit(src_sel=src_sel, dt=dt, g=g, b=b):
                            src = qn[b] if src_sel == 0 else vn[b]
                            dst = qt[b] if src_sel == 0 else vt[b]
                            pst = psc.tile(
                                [128, 512], F32, tag="pb",
                                name=f"pst{b}_{src_sel}_{dt}_{g}",
                            )
                            for j in range(4):
                                blk = g * 4 + j
                                nc.tensor.transpose(
                                    pst[:, j * 128 : (j + 1) * 128],
                                    src[:, blk, dt * 128 : (dt + 1) * 128],
                                    ident[:],
                                )
                            dslice = dst[:, dt, g * 512 : (g + 1) * 512]
                            if copy_eng[0] % 2 == 0:
                                nc.vector.tensor_copy(dslice, pst[:])
                            else:
                                nc.scalar.copy(dslice, pst[:])
                            copy_eng[0] += 1
                        out.append(emit)

            def emit_vh(b=b):
                for kt in range(NKT):
                    nc.vector.tensor_copy(vh[b][:, kt, :], vn[b][:, kt, :])

            out.append(emit_vh)
            return out

        spsum = {}
        s32p = ctx.enter_context(tc.tile_pool(name="s32p", bufs=3))

        last_mm1 = {}

        def emit_mm1(b, qb):
            s = pss.tile([128, LKV], F32, tag="s", name=f"s{b}_{qb}")
            s32 = s32p.tile([128, LKV], F32, tag="s32", name=f"s32_{b}_{qb}")
            halves = []
            for kc in range(2):
                for dt in range(NDT):
                    last_mm1[(b, qb)] = nc.tensor.matmul(
                        s[:, kc * 512 : (kc + 1) * 512],
                        qt[b][:, dt, qb * 128 : (qb + 1) * 128],
                        vt[b][:, dt, kc * 512 : (kc + 1) * 512],
                        start=(dt == 0),
                        stop=(dt == NDT - 1),
                    )
                # evacuate the finished half, then reduce on the SBUF copy
                # (frees the PSUM tile early, enabling the 3-deep lookahead)
                nc.vector.tensor_copy(
                    s32[:, kc * 512 : (kc + 1) * 512],
                    s[:, kc * 512 : (kc + 1) * 512],
                )
                nm = sp.tile([128, 1], F32, tag=f"nm{kc}", name=f"nm{b}_{qb}_{kc}")
                nc.vector.reduce_max(
                    nm[:], s32[:, kc * 512 : (kc + 1) * 512],
                    axis=mybir.AxisListType.X, negate=True,
                )
                halves.append(nm)
            spsum[(b, qb)] = (s, s32, halves)

        chain_state = {}

        def emit_chain(b, qb):
            """Softmax chain up to the transposed E and 1/Z - emitted one slot
            ahead of its consumer so exp/xbar run as early as data allows."""
            s, s32, nmh = spsum.pop((b, qb))
            nm = sp.tile([128, 1], F32, tag="nm", name=f"nm{b}_{qb}")
            nc.vector.tensor_tensor(
                nm[:], nmh[0][:], nmh[1][:], op=mybir.AluOpType.min
            )
            e16 = ep.tile([128, LKV], F16, tag="e16", name=f"e16{b}_{qb}")
            zsum = sp.tile([128, 1], F32, tag="zsum", name=f"zs{b}_{qb}")
            nc.scalar.activation(
                e16[:], s32[:], mybir.ActivationFunctionType.Exp,
                bias=nm[:], scale=1.0, accum_out=zsum[:],
            )
            et = ep.tile([128, NKT, 128], F16, tag="et", name=f"et{b}_{qb}")
            # one xbar transpose per q-tile on the sync ring (concurrent
            # transposes on two rings race in the SDMA xbar and intermittently
            # corrupt the output; split halves pay the ring-transition guard
            # twice)
            nc.sync.dma_start(et[:], e16[:], transpose=True)
            rz = sp.tile([128, 1], F32, tag="rz", name=f"rz{b}_{qb}")
            nc.vector.reciprocal(rz[:], zsum[:])
            chain_state[(b, qb)] = (e16, et, rz)

        def emit_out(b, qb, order_after=None):
            e16, et, rz = chain_state.pop((b, qb))
            u = psc.tile([128, D], F32, tag="pb", name=f"u{b}_{qb}")
            for kt in range(NKT):
                h = nc.tensor.matmul(
                    u[:], et[:, kt, :], vh[b][:, kt, :],
                    start=(kt == 0), stop=(kt == NKT - 1),
                )
                if kt == 0 and order_after is not None:
                    # scheduler hint: keep this waiting matmul head behind two
                    # more ready mm1 groups in the in-order PE stream
                    bass._add_dep_helper(
                        h.ins, order_after.ins, sync=False,
                        reason="mm2-after-mm1-lookahead",
                    )
            a32 = ep.tile([128, LKV], F32, tag="a32", name=f"a32{b}_{qb}")
            nc.scalar.activation(
                a32[:], e16[:], mybir.ActivationFunctionType.Copy, scale=rz[:]
            )
            nc.sync.dma_start(attn_out[b, qb * 128 : (qb + 1) * 128, :], a32[:])
            c32 = ep.tile([128, D], F32, tag="c32", name=f"c32{b}_{qb}")
            nc.vector.tensor_scalar_mul(c32[:], u[:], rz[:])
            nc.sync.dma_start(ctx_out[b, qb * 128 : (qb + 1) * 128, :], c32[:])

        # batch 0's transposes up front; batch 1's are interleaved into batch
        # 0's q-loop as PE filler; global slot pipeline, mm1 3 slots ahead.
        for b in range(BPC):
            alloc_transposed(b)
        groups0 = transpose_groups(0)
        for emit in groups0[:12]:
            emit()
        filler = transpose_groups(1)
        fi = 0
        slots = [(b, qb) for b in range(BPC) for qb in range(NQT)]
        LOOK = 3
        # first q-tiles only need the Q g0 transposes (q < 512) + all of V
        for j in range(LOOK):
            assert slots[j][1] < 4
            emit_mm1(*slots[j])
        emit_chain(*slots[0])
        for emit in groups0[12:]:
            emit()
        for i, (b, qb) in enumerate(slots):
            if i + LOOK < len(slots):
                nb, nqb = slots[i + LOOK]
                # batch-1 score matmuls must be emitted after the filler
                # transposes that produce their operands (Tile tracks RAW
                # deps in emission order)
                assert nb == 0 or fi == len(filler)
                emit_mm1(nb, nqb)
            if fi < len(filler):
                for _ in range(6):
                    if fi < len(filler):
                        filler[fi]()
                        fi += 1
            if i + 1 < len(slots):
                emit_chain(*slots[i + 1])
            oa = slots[i + 2] if i + 2 < len(slots) else None
            emit_out(b, qb, order_after=last_mm1.get(oa) if oa else None)

    _fix_wait_limits(nc)
    return nc


_NC = None


def _get_nc():
    global _NC
    if _NC is None:
        _NC = build()
    return _NC


_IDEN = np.eye(128, dtype=np.float32)


def kernel(query: np.ndarray, value: np.ndarray):
    query = np.ascontiguousarray(query, dtype=np.float32)
    value = np.ascontiguousarray(value, dtype=np.float32)
    nc = _get_nc()
    in_maps = [
        {
            "query": query[c * BPC : (c + 1) * BPC],
            "value": value[c * BPC : (c + 1) * BPC],
            "iden": _IDEN,
        }
        for c in range(N_CORES)
    ]
    res = run_bass_kernel_spmd(nc, in_maps, core_ids=list(range(N_CORES)))
    context = np.concatenate([r["context"] for r in res.results], axis=0)
    attn = np.concatenate([r["attn"] for r in res.results], axis=0)
    return context, attn


# revision 35
# speedup vs baseline: 1.0595x; 1.0595x over previous
"""Dense dot-product attention (score = Q@V^T, softmax, context = A@V) on 8
TRN2 NeuronCores, batch-parallel: each core owns B/8 = 2 batches.

Per batch on one core (Lq = Lkv = 1024, D = 512, fp32 I/O):
  - Q, V loaded in natural [l, d] layout (fast contiguous DMA).
  - QT/VT ([d, l] - the PE contracts over the partition dim) via PE
    transpose-mode matmuls; PSUM->SBUF copies round to float32r, which runs
    matmuls at 1 cycle/row (4x fp32) with ~13-bit mantissa (measured score
    RMS err 3e-3, 16x better than bf16).
  - S = QT.T @ VT accumulated in PSUM per 128-row q-tile; per-half
    reduce_max(negate) overlaps the second half's matmuls.
  - ACT exp(S - max) -> fp16 E with fused row-sum; one DMA xbar
    block-transpose per q-tile (dest[p, kt, j] = E[j, kt*128+p]).
  - context = (ET.T @ V_fp16) * (1/Z); attn = E * (1/Z).
The q-loop runs the score matmul two tiles ahead of its consumer, and the
NEXT batch's PE transposes are interleaved into the current batch's loop as
filler so softmax-chain latency doesn't drain the PE.
"""
import sys

sys.path.insert(0, "/opt/trn_rl_repo")

import collections
from contextlib import ExitStack

import numpy as np

import concourse.bass as bass
import concourse.tile as tile
from concourse import mybir
from concourse.bass_utils import run_bass_kernel_spmd

F32 = mybir.dt.float32
F32R = mybir.dt.float32r
F16 = mybir.dt.float16

N_CORES = 8
B, LQ, LKV, D = 16, 1024, 1024, 512
BPC = B // N_CORES  # batches per core
NQT = LQ // 128
NKT = LKV // 128
NDT = D // 128


# --- post-Tile pass: hardware wait-slot limits -------------------------------
# Engine instructions carry a single hardware semaphore-wait slot; Tile's
# sem-assigner sometimes emits more. Hoist excess waits onto single-wait NOPs
# spliced immediately before the instruction on the same engine (the NX
# sequencer dispatches in order, so the NOPs block until the sems clear).
_WAIT_LIMITS = collections.defaultdict(lambda: 1)


def _fix_wait_limits(nc):
    n_fixed = 0
    for fn in nc.m.functions:
        for blk in fn.blocks:
            out = []
            for inst in blk.instructions:
                limit = _WAIT_LIMITS[type(inst).__name__]
                si = inst.sync_info
                if si is not None and si.on_wait and len(si.on_wait) > limit:
                    hoist = list(si.on_wait)[: len(si.on_wait) - limit]
                    keep = list(si.on_wait)[len(si.on_wait) - limit :]
                    for i, w in enumerate(hoist):
                        out.append(
                            mybir.InstNoOp(
                                name=f"{inst.name}-waitnop{i}",
                                engine=inst.engine,
                                sync_info=mybir.SyncInfo(on_wait=[w], on_update=[]),
                                bass_nofuse=True,
                            )
                        )
                    inst.sync_info = mybir.SyncInfo(
                        on_wait=keep, on_update=list(si.on_update or [])
                    )
                    n_fixed += 1
                out.append(inst)
            blk.instructions = out
    return n_fixed


def build():
    nc = bass.Bass("TRN2", target_bir_lowering=False, debug=False)
    q = nc.dram_tensor("query", [BPC, LQ, D], F32, kind="ExternalInput").ap()
    v = nc.dram_tensor("value", [BPC, LKV, D], F32, kind="ExternalInput").ap()
    iden = nc.dram_tensor("iden", [128, 128], F32, kind="ExternalInput").ap()
    ctx_out = nc.dram_tensor("context", [BPC, LQ, D], F32, kind="ExternalOutput").ap()
    attn_out = nc.dram_tensor("attn", [BPC, LQ, LKV], F32, kind="ExternalOutput").ap()

    with ExitStack() as ctx:
        tc = ctx.enter_context(tile.TileContext(nc))
        singles = ctx.enter_context(tc.tile_pool(name="singles", bufs=1))
        iop = ctx.enter_context(tc.tile_pool(name="io", bufs=2))
        tp = ctx.enter_context(tc.tile_pool(name="tp", bufs=2))
        ep = ctx.enter_context(tc.tile_pool(name="ep", bufs=2))
        sp = ctx.enter_context(tc.tile_pool(name="sp", bufs=8))
        pss = ctx.enter_context(tc.tile_pool(name="pss", bufs=3, space="PSUM"))
        psc = ctx.enter_context(tc.tile_pool(name="psc", bufs=2, space="PSUM"))

        ident = singles.tile([128, 128], F32)
        nc.sync.dma_start(ident[:], iden)

        # all loads issued up front (sync ring) so they never queue behind
        # stores; per-half so the first transposes can start early
        qn, vn = {}, {}
        for b in range(BPC):
            qn[b] = iop.tile([128, NQT, D], F32, tag="qn", name=f"qn{b}")
            vn[b] = iop.tile([128, NKT, D], F32, tag="vn", name=f"vn{b}")
        for b in range(BPC):
            for g in range(2):
                nc.sync.dma_start(
                    qn[b][:, g * 4 : (g + 1) * 4, :],
                    q[b].rearrange("(t p) d -> p t d", p=128)[:, g * 4 : (g + 1) * 4, :],
                )
                nc.sync.dma_start(
                    vn[b][:, g * 4 : (g + 1) * 4, :],
                    v[b].rearrange("(t p) d -> p t d", p=128)[:, g * 4 : (g + 1) * 4, :],
                )

        qt, vt, vh = {}, {}, {}
        copy_eng = [0]

        def alloc_transposed(b):
            vh[b] = tp.tile([128, NKT, D], F16, tag="vh", name=f"vh{b}")
            qt[b] = tp.tile([128, NDT, LQ], F32R, tag="qt", name=f"qt{b}")
            vt[b] = tp.tile([128, NDT, LKV], F32R, tag="vt", name=f"vt{b}")

        def transpose_groups(b):
            """Thunks: each emits one PE-transpose group (4 blocks) + its
            PSUM->SBUF rounding copy; final thunk casts V to fp16."""
            out = []
            for g in range(2):
                for src_sel in (0, 1):
                    for dt in range(NDT):
                        def emit(src_sel=src_sel, dt=dt, g=g, b=b):
                            src = qn[b] if src_sel == 0 else vn[b]
                            dst = qt[b] if src_sel == 0 else vt[b]
                            pst = psc.tile(
                                [128, 512], F32, tag="pb",
                                name=f"pst{b}_{src_sel}_{dt}_{g}",
                            )
                            for j in range(4):
                                blk = g * 4 + j
                                nc.tensor.transpose(
                                    pst[:, j * 128 : (j + 1) * 128],
                                    src[:, blk, dt * 128 : (dt + 1) * 128],
                                    ident[:],
                                )
                            dslice = dst[:, dt, g * 512 : (g + 1) * 512]
                            if copy_eng[0] % 2 == 0:
                                nc.vector.tensor_copy(dslice, pst[:])
                            else:
                                nc.scalar.copy(dslice, pst[:])
                            copy_eng[0] += 1
                        out.append(emit)

            def emit_vh(b=b):
                for kt in range(NKT):
                    nc.vector.tensor_copy(vh[b][:, kt, :], vn[b][:, kt, :])

            out.append(emit_vh)
            return out

        spsum = {}
        s32p = ctx.enter_context(tc.tile_pool(name="s32p", bufs=3))

        last_mm1 = {}

        def emit_mm1(b, qb):
            s = pss.tile([128, LKV], F32, tag="s", name=f"s{b}_{qb}")
            s32 = s32p.tile([128, LKV], F32, tag="s32", name=f"s32_{b}_{qb}")
            halves = []
            for kc in range(2):
                for dt in range(NDT):
                    last_mm1[(b, qb)] = nc.tensor.matmul(
                        s[:, kc * 512 : (kc + 1) * 512],
                        qt[b][:, dt, qb * 128 : (qb + 1) * 128],
                        vt[b][:, dt, kc * 512 : (kc + 1) * 512],
                        start=(dt == 0),
                        stop=(dt == NDT - 1),
                    )
                # evacuate the finished half, then reduce on the SBUF copy
                # (frees the PSUM tile early, enabling the 3-deep lookahead)
                nc.vector.tensor_copy(
                    s32[:, kc * 512 : (kc + 1) * 512],
                    s[:, kc * 512 : (kc + 1) * 512],
                )
                nm = sp.tile([128, 1], F32, tag=f"nm{kc}", name=f"nm{b}_{qb}_{kc}")
                nc.vector.reduce_max(
                    nm[:], s32[:, kc * 512 : (kc + 1) * 512],
                    axis=mybir.AxisListType.X, negate=True,
                )
                halves.append(nm)
            spsum[(b, qb)] = (s, s32, halves)

        def emit_softmax_mm2(b, qb, order_after=None):
            s, s32, nmh = spsum.pop((b, qb))
            nm = sp.tile([128, 1], F32, tag="nm", name=f"nm{b}_{qb}")
            nc.vector.tensor_tensor(
                nm[:], nmh[0][:], nmh[1][:], op=mybir.AluOpType.min
            )
            e16 = ep.tile([128, LKV], F16, tag="e16", name=f"e16{b}_{qb}")
            zsum = sp.tile([128, 1], F32, tag="zsum", name=f"zs{b}_{qb}")
            nc.scalar.activation(
                e16[:], s32[:], mybir.ActivationFunctionType.Exp,
                bias=nm[:], scale=1.0, accum_out=zsum[:],
            )
            et = ep.tile([128, NKT, 128], F16, tag="et", name=f"et{b}_{qb}")
            # one xbar transpose per q-tile on the sync ring (concurrent
            # transposes on two rings race in the SDMA xbar and intermittently
            # corrupt the output; split halves pay the ring-transition guard
            # twice)
            nc.sync.dma_start(et[:], e16[:], transpose=True)
            rz = sp.tile([128, 1], F32, tag="rz", name=f"rz{b}_{qb}")
            nc.vector.reciprocal(rz[:], zsum[:])
            u = psc.tile([128, D], F32, tag="pb", name=f"u{b}_{qb}")
            for kt in range(NKT):
                h = nc.tensor.matmul(
                    u[:], et[:, kt, :], vh[b][:, kt, :],
                    start=(kt == 0), stop=(kt == NKT - 1),
                )
                if kt == 0 and order_after is not None:
                    # scheduler hint: keep this waiting matmul head behind two
                    # more ready mm1 groups in the in-order PE stream
                    bass._add_dep_helper(
                        h.ins, order_after.ins, sync=False,
                        reason="mm2-after-mm1-lookahead",
                    )
            a32 = ep.tile([128, LKV], F32, tag="a32", name=f"a32{b}_{qb}")
            nc.scalar.activation(
                a32[:], e16[:], mybir.ActivationFunctionType.Copy, scale=rz[:]
            )
            nc.sync.dma_start(attn_out[b, qb * 128 : (qb + 1) * 128, :], a32[:])
            c32 = ep.tile([128, D], F32, tag="c32", name=f"c32{b}_{qb}")
            nc.vector.tensor_scalar_mul(c32[:], u[:], rz[:])
            nc.sync.dma_start(ctx_out[b, qb * 128 : (qb + 1) * 128, :], c32[:])

        # batch 0's transposes up front; batch 1's are interleaved into batch
        # 0's q-loop as PE filler; global slot pipeline, mm1 3 slots ahead.
        for b in range(BPC):
            alloc_transposed(b)
        for emit in transpose_groups(0):
            emit()
        filler = transpose_groups(1)
        fi = 0
        slots = [(b, qb) for b in range(BPC) for qb in range(NQT)]
        LOOK = 3
        for j in range(LOOK):
            emit_mm1(*slots[j])
        for i, (b, qb) in enumerate(slots):
            if i + LOOK < len(slots):
                nb, nqb = slots[i + LOOK]
                # batch-1 score matmuls must be emitted after the filler
                # transposes that produce their operands (Tile tracks RAW
                # deps in emission order)
                assert nb == 0 or fi == len(filler)
                emit_mm1(nb, nqb)
            if fi < len(filler):
                for _ in range(6):
                    if fi < len(filler):
                        filler[fi]()
                        fi += 1
            oa = slots[i + 2] if i + 2 < len(slots) else None
            emit_softmax_mm2(b, qb, order_after=last_mm1.get(oa) if oa else None)

    _fix_wait_limits(nc)
    return nc


_NC = None


def _get_nc():
    global _NC
    if _NC is None:
        _NC = build()
    return _NC


_IDEN = np.eye(128, dtype=np.float32)


def kernel(query: np.ndarray, value: np.ndarray):
    query = np.ascontiguousarray(query, dtype=np.float32)
    value = np.ascontiguousarray(value, dtype=np.float32)
    nc = _get_nc()
    in_maps = [
        {
            "query": query[c * BPC : (c + 1) * BPC],
            "value": value[c * BPC : (c + 1) * BPC],
            "iden": _IDEN,
        }
        for c in range(N_CORES)
    ]
    res = run_bass_kernel_spmd(nc, in_maps, core_ids=list(range(N_CORES)))
    context = np.concatenate([r["context"] for r in res.results], axis=0)
    attn = np.concatenate([r["attn"] for r in res.results], axis=0)
    return context, attn
